# revision 29
# baseline (speedup 1.0000x reference)
"""Trainium2 Bass kernel for masked multi-modal causal dot-product attention.

Computation (reference):
  Q = mlp(x1, Wq)               # (4096, 64), 3 linear layers, relu between
  for m in 0..3:
    K_m = mlp(x_m, Wk[m])       # (4096, 64)
    mask_m[i,j] = t2_m[j] <= t1[i]   (timestamps sorted -> staircase mask)
    acc += ((Q @ K_m.T) * mask_m) @ x_m[:, :2]
  out = acc  # (1, 4096, 2)

Sharding: 8 cores = 4 modalities x 2 contiguous query halves (2048 queries
each). One SPMD program; per-core variation lives in the input tensors.

Key algebraic optimization: for key tiles FULLY visible to a whole query
block, ((Q K^T) * 1) V = Q (K^T V). Per 128-key pair tile j we form
G_j^T = V_j^T K_j (2x64) with two tiny matmuls:
  - probe:  sp = kTblk_j^T @ [I64; I64]  -> K values, keys on partitions
  - reduce: G_j^T = xkv_j^T @ sp         -> PSUM (2, 64)
G tiles are staged to SBUF and DMA'd into a [32, 2, 64] partition-stacked
array; a step-vector matmul (host-built, per-core data) then selects the
prefix sum G_pref_b = sum_{j < F[b]} G_j for each query block -> the whole
fully-visible region costs ONE 512-col matmul per block. Only the ~5 boundary
tiles per block (keys whose timestamp falls inside the block's time span) run
the explicit S -> fused mask-multiply (DVE scalar_tensor_tensor) -> AV path.
Boundary keys are host-gathered into fixed tile slots so a single program
serves all cores; padded slots use t2=+inf and mask to zero.

Packing (from baseline): feature dim 64 is packed to contraction 128
everywhere (block-diagonal MLP weights on stacked halves, block-diagonal
K^T pair tiles, Q^T replicated onto both partition halves). All matmuls f32r.
"""

import os
import sys

import numpy as np
import ml_dtypes

BF16 = ml_dtypes.bfloat16

sys.path.insert(0, "/opt/trn_rl_repo")

T = 4096
D = 64
M = 4
NLIN = 3
NQ = 2048          # queries per core (contiguous half)
CHUNK = 128        # keys per pair tile (64 even + 64 odd)
NPAIR = T // CHUNK  # 32 sorted pair tiles
IBLK = 512         # query block (moving dim)
NBLK = NQ // IBLK  # 4 query blocks per core

LAST_RESULTS = None


def _build_program(NBB):
    """NBB[b]: boundary slots for query block b (same for all cores; per-core
    variation is in the gathered input data)."""
    import concourse.bacc as bacc
    import concourse.mybir as mybir
    import concourse.tile as tile

    f32 = mybir.dt.float32
    f32r = mybir.dt.float32r
    bf16 = mybir.dt.bfloat16
    Relu = mybir.ActivationFunctionType.Relu
    Identity = mybir.ActivationFunctionType.Identity
    is_ge = mybir.AluOpType.is_ge
    add = mybir.AluOpType.add
    amax = mybir.AluOpType.max
    mult = mybir.AluOpType.mult

    NBSLOT = sum(NBB)             # total boundary slots
    NSLOT = NPAIR + NBSLOT        # total pair tiles in kTblk
    KCOLS = NSLOT * 64            # K-MLP moving columns
    boff = [NPAIR + sum(NBB[:b]) for b in range(NBLK)]  # first slot of block b

    nc = bacc.Bacc("TRN2", target_bir_lowering=False, debug=False, num_devices=8)

    xqT = nc.dram_tensor("xqT", [128, NQ // 2], bf16, kind="ExternalInput")
    xkT = nc.dram_tensor("xkT", [128, KCOLS], bf16, kind="ExternalInput")
    xkvD = nc.dram_tensor("xkvD", [128, NPAIR * 64], bf16, kind="ExternalInput")
    xkvB = nc.dram_tensor("xkvB", [128, max(NBSLOT, 1) * 4], bf16, kind="ExternalInput")
    xt2b = nc.dram_tensor("xt2b", [128, max(NBSLOT, 1)], f32, kind="ExternalInput")
    t1p = nc.dram_tensor("t1p", [1, NQ], f32, kind="ExternalInput")
    probe = nc.dram_tensor("probe", [128, 64], bf16, kind="ExternalInput")
    stepm = nc.dram_tensor("stepm", [64, 2 * NBLK], bf16, kind="ExternalInput")
    wq = nc.dram_tensor("wq", [128, 4 * 128], bf16, kind="ExternalInput")
    bq = nc.dram_tensor("bq", [128, 4], f32, kind="ExternalInput")
    wk = nc.dram_tensor("wk", [128, NLIN * 128], bf16, kind="ExternalInput")
    bk = nc.dram_tensor("bk", [128, NLIN], f32, kind="ExternalInput")
    out = nc.dram_tensor("out", [2, NQ], f32, kind="ExternalOutput")

    def rr(ap):
        return ap.bitcast(f32r)

    with tile.TileContext(nc) as tc:
        with (
            tc.tile_pool(name="const", bufs=1) as const,
            tc.tile_pool(name="hq", bufs=2) as hqp,
            tc.tile_pool(name="hk", bufs=2) as hkp,
            tc.tile_pool(name="spool", bufs=3) as spool,
            tc.tile_pool(name="gpool", bufs=4) as gpool,
            tc.tile_pool(name="ps_a", bufs=2, space="PSUM") as ps_a,
            tc.tile_pool(name="ps_s", bufs=2, space="PSUM") as ps_s,
            tc.tile_pool(name="ps_g", bufs=2, space="PSUM") as ps_g,
            tc.tile_pool(name="ps_ov", bufs=2, space="PSUM") as ps_ov,
        ):
            # ---- inputs -> SBUF (weights first, x chunked for overlap)
            wk_sb = const.tile([128, NLIN, 128], bf16)
            nc.sync.dma_start(wk_sb[:], wk[:].rearrange("p (l e) -> p l e", l=NLIN))
            xkT_sb = const.tile([128, KCOLS], bf16)
            xqT_sb = const.tile([128, NQ // 2], bf16)
            t1b_sb = const.tile([CHUNK, NQ], f32)
            nchk = -(-KCOLS // IBLK)

            def kchunk_dma(eng, c):
                sl = slice(c * IBLK, min((c + 1) * IBLK, KCOLS))
                eng.dma_start(xkT_sb[:, sl], xkT[:, sl])

            kchunk_dma(nc.sync, 4)
            bk_sb = const.tile([128, NLIN], f32)
            nc.sync.dma_start(bk_sb[:], bk[:])
            nc.sync.dma_start(xqT_sb[:, 0:IBLK], xqT[:, 0:IBLK])
            nc.sync.dma_start(xqT_sb[:, IBLK:2 * IBLK], xqT[:, IBLK:2 * IBLK])
            kchunk_dma(nc.sync, 0)
            nc.sync.dma_start(t1b_sb[:], t1p[:].partition_broadcast(CHUNK))
            for c in (5, 1, 6, 2, 3):
                kchunk_dma(nc.sync, c)
            wq_sb = const.tile([128, 4, 128], bf16)
            nc.gpsimd.dma_start(wq_sb[:], wq[:].rearrange("p (l e) -> p l e", l=4))
            bq_sb = const.tile([128, 4], f32)
            nc.gpsimd.dma_start(bq_sb[:], bq[:])
            xt2b_sb = const.tile([128, max(NBSLOT, 1)], f32)
            nc.gpsimd.dma_start(xt2b_sb[:], xt2b[:])
            xkvB_sb = const.tile([128, max(NBSLOT, 1), 4], bf16)
            nc.gpsimd.dma_start(xkvB_sb[:], xkvB[:].rearrange("p (c f) -> p c f", f=4))
            probe_sb = const.tile([128, 64], bf16)
            nc.gpsimd.dma_start(probe_sb[:], probe[:])
            xkvD_sb = const.tile([128, NPAIR, 64], bf16)
            nc.gpsimd.dma_start(xkvD_sb[:], xkvD[:].rearrange("p (c f) -> p c f", f=64))
            step_sb = const.tile([64, 2 * NBLK], bf16)
            nc.gpsimd.dma_start(step_sb[:], stepm[:])
            outA_sb = const.tile([4, IBLK], f32)
            outB_sb = const.tile([4, IBLK], f32)

            # ---- blocked K^T target: pair tiles with block-diagonal layout
            kTblk = const.tile([128, NSLOT, CHUNK], bf16)
            nc.vector.memset(kTblk[0:64, :, 64:128], 0.0)
            nc.vector.memset(kTblk[64:128, :, 0:64], 0.0)
            qT2 = const.tile([128, NQ], bf16)
            G_all_sb = const.tile([64, 64], bf16)
            gstat = const.tile([128, NBLK, 4], bf16)
            nc.vector.memset(gstat[:], 0.0)

            # ---- stacked MLPs (block-diagonal weights, both halves at once)
            def epilogue(dst, ps, bias, layer, eng):
                if eng == "act":
                    func = Relu if layer < NLIN - 1 else Identity
                    nc.scalar.activation(dst, ps, func, bias=bias)
                elif layer < NLIN - 1:
                    nc.vector.tensor_scalar(dst, ps, bias, 0.0, op0=add, op1=amax)
                else:
                    nc.vector.tensor_scalar(dst, ps, bias, None, op0=add)

            hk1 = hkp.tile([128, KCOLS], bf16, tag="h")
            hk2 = hkp.tile([128, KCOLS], bf16, tag="h")
            hq1 = hqp.tile([128, NQ // 2], bf16, tag="h")
            hq2 = hqp.tile([128, NQ // 2], bf16, tag="h")

            def k_stage(c, layer, eng="act"):
                sl = slice(c * IBLK, min((c + 1) * IBLK, KCOLS))
                csz = sl.stop - sl.start
                cur = xkT_sb if layer == 0 else hk1
                dst = hk1 if layer == 0 else hk2
                ps = ps_a.tile([128, csz], f32, tag="a", name="ps")
                nc.tensor.matmul(
                    ps[:], wk_sb[:, layer, :], cur[:, sl], start=True, stop=True
                )
                epilogue(dst[:, sl], ps[:], bk_sb[:, layer:layer + 1], layer, eng)

            eng_flip = 0

            def k_final(c, act_only=False):
                nonlocal eng_flip
                sl = slice(c * IBLK, min((c + 1) * IBLK, KCOLS))
                csz = sl.stop - sl.start
                ps = ps_a.tile([128, csz], f32, tag="a", name="ps")
                nc.tensor.matmul(
                    ps[:], wk_sb[:, NLIN - 1, :], hk2[:, sl], start=True, stop=True
                )
                psv = ps[:].rearrange("p (a e) -> p a e", e=64)
                pair = slice(8 * c, 8 * c + csz // 64)
                bias = bk_sb[:, NLIN - 1 : NLIN]
                for half, csl in ((slice(0, 64), slice(0, 64)),
                                  (slice(64, 128), slice(64, 128))):
                    dst = kTblk[half, pair, csl]
                    srcv = psv[half, :, :]
                    if act_only or eng_flip % 2 == 0:
                        nc.scalar.activation(dst, srcv, Identity, bias=bias[half])
                    else:
                        nc.vector.tensor_scalar(dst, srcv, bias[half], None, op0=add)
                    eng_flip += 1

            def q_stage(layer, nb, eng="dve"):
                sl = slice(nb * IBLK, (nb + 1) * IBLK)
                cur = xqT_sb if layer == 0 else hq1
                dst = hq1 if layer == 0 else hq2
                ps = ps_a.tile([128, IBLK], f32, tag="a", name="ps")
                nc.tensor.matmul(
                    ps[:], wq_sb[:, layer, :], cur[:, sl], start=True, stop=True
                )
                epilogue(dst[:, sl], ps[:], bq_sb[:, layer:layer + 1], layer, eng)

            def q_final(nb, rep):
                sl = slice(nb * IBLK, (nb + 1) * IBLK)
                bias = bq_sb[:, NLIN - 1 : NLIN]
                ps = ps_a.tile([128, IBLK], f32, tag="a", name="ps")
                nc.tensor.matmul(
                    ps[:], wq_sb[:, 2 + rep, :], hq2[:, sl], start=True, stop=True
                )
                osl = slice(rep * (NQ // 2) + nb * IBLK,
                            rep * (NQ // 2) + (nb + 1) * IBLK)
                epilogue(qT2[:, osl], ps[:], bias, NLIN - 1,
                         "act" if rep else "dve")

            # ---- G machinery: two output banks (blocks 0-1 / 2-3)
            ovA = ps_ov.tile([4, IBLK], f32, tag="ov")
            ovB = ps_ov.tile([4, IBLK], f32, tag="ov")
            gall_ps = ps_g.tile([64, 64], f32, tag="g")
            sps_tiles = {}

            def job_probe_s(r):
                spb = ps_a.tile([128, 512], f32, tag="a", name="spb")
                for slq in range(8):
                    j = r * 8 + slq
                    nc.tensor.matmul(
                        spb[:, slq * 64:(slq + 1) * 64], kTblk[:, j, :],
                        probe_sb[:], start=True, stop=True, skip_group_check=True,
                    )
                sps = gpool.tile([128, 512], bf16, name="sps")
                nc.scalar.copy(sps[:], spb[:])
                sps_tiles[r] = sps

            def job_probe_av(r):
                sps = sps_tiles[r]
                for slq in range(8):
                    j = r * 8 + slq
                    nc.tensor.matmul(
                        gall_ps[:], xkvD_sb[:, j, :],
                        sps[:, slq * 64:(slq + 1) * 64],
                        start=(j == 0), stop=(j == NPAIR - 1),
                        skip_group_check=True,
                    )
                if r == NPAIR // 8 - 1:
                    nc.scalar.copy(G_all_sb[:], gall_ps[:])

            def job_select():
                psel = ps_g.tile([64, 2 * NBLK], f32, tag="g")
                nc.tensor.matmul(
                    psel[:], G_all_sb[:], step_sb[:],
                    start=True, stop=True, skip_group_check=True,
                )
                for b in range(NBLK):
                    for c in range(2):
                        i = c * NBLK + b
                        o = 2 * (b % 2) + c
                        nc.scalar.copy(gstat[0:64, b, o:o + 1],
                                       psel[:, i:i + 1])

            def job_full(b, stop=False):
                isl = slice(b * IBLK, (b + 1) * IBLK)
                ov = ovA if b < 2 else ovB
                nc.tensor.matmul(
                    ov[:], gstat[:, b, :], qT2[:, isl],
                    start=False, stop=stop, skip_group_check=True,
                )

            def readout(bank):
                ov = ovA if bank == 0 else ovB
                osb = outA_sb if bank == 0 else outB_sb
                nc.scalar.copy(osb[:], ov[:])
                for b in ((0, 1) if bank == 0 else (2, 3)):
                    isl = slice(b * IBLK, (b + 1) * IBLK)
                    eng = nc.sync if b % 2 == 0 else nc.gpsimd
                    o = 2 * (b % 2)
                    eng.dma_start(out[:, isl], osb[o:o + 2, :])

            # ---- HAM warm-up: dead matmuls on a memset stationary start
            # right after the preamble (no DMA dependency) so the PE clock is
            # at 8/8 when the real stream begins
            warmw = const.tile([128, 128], bf16)
            nc.vector.memset(warmw[:], 0.0)
            for w in range(24):
                wps = ps_s.tile([128, 128], f32, tag="s", name="wps")
                nc.tensor.matmul(
                    wps[:], warmw[:], warmw[:],
                    start=True, stop=True, skip_group_check=True,
                )

            # ---- critical chain: chunk 4 K + Q chain -> first S/mask asap
            k_stage(4, 0)
            q_stage(0, 0)
            q_stage(0, 1)
            k_stage(4, 1)
            q_stage(1, 0)
            q_stage(1, 1)
            k_final(4)
            q_final(0, 0)

            # ---- jobs: deadline list (sorted) + filler list, interleaved
            seq = [(b, s) for b in range(NBLK) for s in range(NBB[b])]

            def first_it_with_slot_ge(slot0):
                for i, (b, s) in enumerate(seq):
                    if boff[b] + s >= slot0:
                        return i
                return len(seq)

            def first_it_of_block(b0):
                for i, (b, s) in enumerate(seq):
                    if b == b0:
                        return i
                return len(seq)

            d5 = first_it_with_slot_ge(NPAIR + 8)
            d6 = first_it_with_slot_ge(NPAIR + 16)
            dB1 = first_it_of_block(1)
            dB2 = first_it_of_block(2)
            dB3 = first_it_of_block(3)

            dlist = sorted([
                (max(d5 - 2, 0), 0, lambda: k_stage(5, 0)),
                (max(d5 - 1, 0), 1, lambda: k_stage(5, 1)),
                (d5, 2, lambda: k_final(5)),
                (dB1, 3, lambda: q_final(1, 0)),
                (max(d6 - 2, 0), 4, lambda: k_stage(6, 0)),
                (max(d6 - 1, 0), 5, lambda: k_stage(6, 1)),
                (d6, 6, lambda: k_final(6, True)),
                (dB2, 7, lambda: q_final(0, 1)),
                (dB3, 8, lambda: q_final(1, 1)),
            ], key=lambda x: (x[0], x[1]))
            flist = [
                lambda: k_stage(0, 0),
                lambda: k_stage(0, 1),
                lambda: (k_final(0, True), job_probe_s(0)),
                lambda: k_stage(1, 0),
                lambda: k_stage(1, 1),
                lambda: (k_final(1, True), job_probe_s(1), job_probe_av(0)),
                lambda: k_stage(2, 0),
                lambda: k_stage(2, 1),
                lambda: (k_final(2, True), job_probe_s(2), job_probe_av(1)),
                lambda: k_stage(3, 0),
                lambda: k_stage(3, 1),
                lambda: (k_final(3, True), job_probe_s(3), job_probe_av(2)),
                lambda: job_probe_av(NPAIR // 8 - 1),
                lambda: job_select(),
                lambda: job_full(0),
                lambda: (job_full(1, stop=True), readout(0)),
                lambda: job_full(2),
                lambda: job_full(3),
            ]

            # ---- main interleave: jobs first, then S -> mask -> AV(deferred)
            first_av_b23 = next(i for i, (b, s) in enumerate(
                [(b, s) for b in range(NBLK) for s in range(NBB[b])]) if b == 2)

            def emit_av(b, s, first, last):
                slot = boff[b] + s
                bidx = slot - NPAIR
                ov = ovA if b < 2 else ovB
                nc.tensor.matmul(
                    ov[:], xkvB_sb[:, bidx, :], s_tiles[(b, s)][:],
                    start=first, stop=last, skip_group_check=True,
                )

            s_tiles = {}
            di = fi = 0
            prev_av = None
            for it, (b, s) in enumerate(seq):
                while di < len(dlist) and dlist[di][0] <= it:
                    dlist[di][2]()
                    di += 1
                while fi < len(flist) and di + fi < (it * 13) // 10 + 1:
                    flist[fi]()
                    fi += 1
                slot = boff[b] + s
                bidx = slot - NPAIR
                isl = slice(b * IBLK, (b + 1) * IBLK)
                sp = ps_s.tile([CHUNK, IBLK], f32, tag="s", name="sp")
                nc.tensor.matmul(
                    sp[:], kTblk[:, slot, :], qT2[:, isl],
                    start=True, stop=True, skip_group_check=True,
                )
                s_sb = spool.tile([CHUNK, IBLK], bf16, name="s_sb")
                nc.vector.scalar_tensor_tensor(
                    s_sb[:], t1b_sb[:, isl], xt2b_sb[:, bidx:bidx + 1], sp[:],
                    op0=is_ge, op1=mult,
                )
                s_tiles[(b, s)] = s_sb
                if prev_av is not None:
                    emit_av(*prev_av,
                            prev_av == seq[0] or prev_av == seq[first_av_b23],
                            False)
                prev_av = (b, s)
            while di < len(dlist):
                dlist[di][2]()
                di += 1
            while fi < len(flist):
                flist[fi]()
                fi += 1
            emit_av(*prev_av, False, True)
            readout(1)

    nc.compile()
    return nc


def kernel(x1, x2, x3, x4, Wq_w, Wq_b, Wk_w, Wk_b):
    from concourse.bass_utils import run_bass_kernel_spmd

    global LAST_RESULTS

    xs = [np.asarray(a, dtype=np.float32)[0, 0] for a in (x1, x2, x3, x4)]
    Wq_w = np.asarray(Wq_w, dtype=np.float32)
    Wq_b = np.asarray(Wq_b, dtype=np.float32)
    Wk_w = np.asarray(Wk_w, dtype=np.float32)
    Wk_b = np.asarray(Wk_b, dtype=np.float32)

    t1 = xs[0][:, -1]
    t2s = [x[:, -1] for x in xs]

    # ---- per-core full/boundary classification (exact, from timestamps)
    FJ = {}  # (m, p) -> (F[b], J[b])
    NBB = [1] * NBLK
    for p in range(2):
        qoff = NQ * p
        for m in range(M):
            F, J = [], []
            for b in range(NBLK):
                lo = t1[qoff + b * IBLK]
                hi = t1[qoff + b * IBLK + IBLK - 1]
                nfull = int(np.searchsorted(t2s[m], lo, side="right"))
                nvis = int(np.searchsorted(t2s[m], hi, side="right"))
                F.append(nfull // CHUNK)
                J.append(-(-nvis // CHUNK))
                NBB[b] = max(NBB[b], J[b] - F[b])
            FJ[(m, p)] = (F, J)

    nc = _build_program(NBB)

    NBSLOT = sum(NBB)
    boff = [sum(NBB[:b]) for b in range(NBLK)]

    # ---- host packing
    def blockdiag(Wl):
        b = np.zeros((128, 128), np.float32)
        b[:64, :64] = Wl
        b[64:, 64:] = Wl
        return b

    # Q weights: layers 0,1 blockdiag; final as [[W,W],[0,0]] and [[0,0],[W,W]]
    wq_h = np.zeros((4, 128, 128), np.float32)
    for l in range(NLIN - 1):
        wq_h[l] = blockdiag(Wq_w[l])
    wq_h[2, :64, :64] = Wq_w[2]
    wq_h[2, :64, 64:] = Wq_w[2]
    wq_h[3, 64:, :64] = Wq_w[2]
    wq_h[3, 64:, 64:] = Wq_w[2]
    wq_h = np.ascontiguousarray(wq_h.transpose(1, 0, 2).reshape(128, 4 * 128))
    bq_h = np.tile(Wq_b.T, (2, 1))  # [128, 3]
    bq_h = np.ascontiguousarray(
        np.concatenate([bq_h, bq_h[:, 2:3]], axis=1)
    )  # [128, 4]

    probe_h = np.ascontiguousarray(
        np.concatenate([np.eye(64, dtype=np.float32)] * 2, axis=0)
    )  # [128, 64]

    x1T = np.ascontiguousarray(xs[0].T)

    def pack_tile(xrows):
        """[128, D] key rows -> ([128, 64] xkT block, [128, 2] V, [128] t2)."""
        ev, od = xrows[0:64], xrows[64:128]
        blk = np.concatenate([ev.T, od.T], axis=0)  # [128, 64]
        v = np.concatenate([ev[:, 0:2], od[:, 0:2]], axis=0)  # [128, 2]
        tt = np.concatenate([ev[:, -1], od[:, -1]], axis=0)  # [128]
        return blk, v, tt

    in_maps = []
    for c in range(8):
        m, p = c // 2, c % 2
        xm = xs[m]
        qoff = NQ * p
        F, J = FJ[(m, p)]

        NSLOT = NPAIR + NBSLOT
        xkT_h = np.zeros((128, NSLOT * 64), np.float32)
        xkvD_h = np.zeros((128, NPAIR, 64), np.float32)
        xkvB_h = np.zeros((128, max(NBSLOT, 1), 4), np.float32)
        xt2b_h = np.full((128, max(NBSLOT, 1)), 1e30, np.float32)
        for j in range(NPAIR):
            blk, v, tt = pack_tile(xm[CHUNK * j:CHUNK * (j + 1)])
            xkT_h[:, 64 * j:64 * (j + 1)] = blk
            xkvD_h[:, j, 2 * j: 2 * j + 2] = v
        for b in range(NBLK):
            for s in range(NBB[b]):
                t = F[b] + s
                slot = NPAIR + boff[b] + s
                if t < J[b]:
                    blk, v, tt = pack_tile(xm[CHUNK * t:CHUNK * (t + 1)])
                    xkT_h[:, 64 * slot:64 * (slot + 1)] = blk
                    if not os.environ.get('DBG_NO_BND'):
                        o = 2 * (b % 2)
                        xkvB_h[:, boff[b] + s, o:o + 2] = v
                    xt2b_h[:, boff[b] + s] = tt
        step_h = np.zeros((64, 2 * NBLK), np.float32)
        if not os.environ.get('DBG_NO_FULL'):
            for b in range(NBLK):
                for c in range(2):
                    for j in range(F[b]):
                        step_h[2 * j + c, c * NBLK + b] = 1.0

        wk_h = np.stack([blockdiag(Wk_w[m][l]) for l in range(NLIN)])
        wk_h = np.ascontiguousarray(wk_h.transpose(1, 0, 2).reshape(128, NLIN * 128))
        bk_h = np.ascontiguousarray(np.tile(Wk_b[m].T, (2, 1)))  # [128, 3]

        # query-side: contiguous half, [first 1024 | second 1024] stacking
        xq = x1T[:, qoff:qoff + NQ]  # [64, 2048]
        xqT_h = np.concatenate([xq[:, : NQ // 2], xq[:, NQ // 2:]], axis=0)

        in_maps.append(
            {
                "xqT": np.ascontiguousarray(xqT_h).astype(BF16),
                "xkT": xkT_h.astype(BF16),
                "xkvD": np.ascontiguousarray(xkvD_h.reshape(128, NPAIR * 64)).astype(BF16),
                "xkvB": np.ascontiguousarray(
                    xkvB_h.reshape(128, max(NBSLOT, 1) * 4)).astype(BF16),
                "xt2b": xt2b_h,
                "t1p": np.ascontiguousarray(t1[qoff:qoff + NQ][None, :]),
                "probe": probe_h.astype(BF16),
                "stepm": step_h.astype(BF16),
                "wq": wq_h.astype(BF16),
                "bq": bq_h,
                "wk": wk_h.astype(BF16),
                "bk": bk_h,
            }
        )

    res = run_bass_kernel_spmd(nc, in_maps, core_ids=list(range(8)))
    LAST_RESULTS = res

    # ---- gather: sum over modalities per contiguous half, transpose
    acc = np.zeros((2, T), dtype=np.float32)
    for c in range(8):
        m, p = c // 2, c % 2
        acc[:, NQ * p:NQ * (p + 1)] += res.results[c]["out"]
    return np.ascontiguousarray(acc.T)[None]


# revision 30
# speedup vs baseline: 1.0666x; 1.0666x over previous
"""Trainium2 Bass kernel for masked multi-modal causal dot-product attention.

Computation (reference):
  Q = mlp(x1, Wq)               # (4096, 64), 3 linear layers, relu between
  for m in 0..3:
    K_m = mlp(x_m, Wk[m])       # (4096, 64)
    mask_m[i,j] = t2_m[j] <= t1[i]   (timestamps sorted -> staircase mask)
    acc += ((Q @ K_m.T) * mask_m) @ x_m[:, :2]
  out = acc  # (1, 4096, 2)

Sharding: 8 cores = 4 modalities x 2 contiguous query halves (2048 queries
each). One SPMD program; per-core variation lives in the input tensors.

Key algebraic optimization: for key tiles FULLY visible to a whole query
block, ((Q K^T) * 1) V = Q (K^T V). Per 128-key pair tile j we form
G_j^T = V_j^T K_j (2x64) with two tiny matmuls:
  - probe:  sp = kTblk_j^T @ [I64; I64]  -> K values, keys on partitions
  - reduce: G_j^T = xkv_j^T @ sp         -> PSUM (2, 64)
G tiles are staged to SBUF and DMA'd into a [32, 2, 64] partition-stacked
array; a step-vector matmul (host-built, per-core data) then selects the
prefix sum G_pref_b = sum_{j < F[b]} G_j for each query block -> the whole
fully-visible region costs ONE 512-col matmul per block. Only the ~5 boundary
tiles per block (keys whose timestamp falls inside the block's time span) run
the explicit S -> fused mask-multiply (DVE scalar_tensor_tensor) -> AV path.
Boundary keys are host-gathered into fixed tile slots so a single program
serves all cores; padded slots use t2=+inf and mask to zero.

Packing (from baseline): feature dim 64 is packed to contraction 128
everywhere (block-diagonal MLP weights on stacked halves, block-diagonal
K^T pair tiles, Q^T replicated onto both partition halves). All matmuls f32r.
"""

import os
import sys

import numpy as np
import ml_dtypes

BF16 = ml_dtypes.bfloat16

sys.path.insert(0, "/opt/trn_rl_repo")

T = 4096
D = 64
M = 4
NLIN = 3
NQ = 2048          # queries per core (contiguous half)
CHUNK = 128        # keys per pair tile (64 even + 64 odd)
NPAIR = T // CHUNK  # 32 sorted pair tiles
IBLK = 512         # query block (moving dim)
NBLK = NQ // IBLK  # 4 query blocks per core

LAST_RESULTS = None


def _build_program(NBB):
    """NBB[b]: boundary slots for query block b (same for all cores; per-core
    variation is in the gathered input data)."""
    import concourse.bacc as bacc
    import concourse.mybir as mybir
    import concourse.tile as tile

    f32 = mybir.dt.float32
    f32r = mybir.dt.float32r
    bf16 = mybir.dt.bfloat16
    Relu = mybir.ActivationFunctionType.Relu
    Identity = mybir.ActivationFunctionType.Identity
    is_ge = mybir.AluOpType.is_ge
    add = mybir.AluOpType.add
    amax = mybir.AluOpType.max
    mult = mybir.AluOpType.mult

    NBSLOT = sum(NBB)             # total boundary slots
    NSLOT = NPAIR + NBSLOT        # total pair tiles in kTblk
    KCOLS = NSLOT * 64            # K-MLP moving columns
    boff = [NPAIR + sum(NBB[:b]) for b in range(NBLK)]  # first slot of block b

    nc = bacc.Bacc("TRN2", target_bir_lowering=False, debug=False, num_devices=8)

    xqT = nc.dram_tensor("xqT", [128, NQ // 2], bf16, kind="ExternalInput")
    xkT = nc.dram_tensor("xkT", [128, KCOLS], bf16, kind="ExternalInput")
    xkvD = nc.dram_tensor("xkvD", [128, NPAIR * 64], bf16, kind="ExternalInput")
    xkvB = nc.dram_tensor("xkvB", [128, max(NBSLOT, 1) * 4], bf16, kind="ExternalInput")
    xt2b = nc.dram_tensor("xt2b", [128, max(NBSLOT, 1)], f32, kind="ExternalInput")
    t1p = nc.dram_tensor("t1p", [1, NQ], f32, kind="ExternalInput")
    probe = nc.dram_tensor("probe", [128, 64], bf16, kind="ExternalInput")
    stepm = nc.dram_tensor("stepm", [64, 2 * NBLK], bf16, kind="ExternalInput")
    wq = nc.dram_tensor("wq", [128, 4 * 128], bf16, kind="ExternalInput")
    bq = nc.dram_tensor("bq", [128, 4], f32, kind="ExternalInput")
    wk = nc.dram_tensor("wk", [128, NLIN * 128], bf16, kind="ExternalInput")
    bk = nc.dram_tensor("bk", [128, NLIN], f32, kind="ExternalInput")
    out = nc.dram_tensor("out", [2, NQ], f32, kind="ExternalOutput")

    def rr(ap):
        return ap.bitcast(f32r)

    with tile.TileContext(nc) as tc:
        with (
            tc.tile_pool(name="const", bufs=1) as const,
            tc.tile_pool(name="hq", bufs=2) as hqp,
            tc.tile_pool(name="hk", bufs=2) as hkp,
            tc.tile_pool(name="spool", bufs=3) as spool,
            tc.tile_pool(name="gpool", bufs=4) as gpool,
            tc.tile_pool(name="ps_a", bufs=3, space="PSUM") as ps_a,
            tc.tile_pool(name="ps_s", bufs=2, space="PSUM") as ps_s,
            tc.tile_pool(name="ps_g", bufs=1, space="PSUM") as ps_g,
            tc.tile_pool(name="ps_ov", bufs=2, space="PSUM") as ps_ov,
        ):
            # ---- inputs -> SBUF (weights first, x chunked for overlap)
            wk_sb = const.tile([128, NLIN, 128], bf16)
            nc.sync.dma_start(wk_sb[:], wk[:].rearrange("p (l e) -> p l e", l=NLIN))
            xkT_sb = const.tile([128, KCOLS], bf16)
            xqT_sb = const.tile([128, NQ // 2], bf16)
            t1b_sb = const.tile([CHUNK, NQ], f32)
            nchk = -(-KCOLS // IBLK)

            def kchunk_dma(eng, c):
                sl = slice(c * IBLK, min((c + 1) * IBLK, KCOLS))
                eng.dma_start(xkT_sb[:, sl], xkT[:, sl])

            kchunk_dma(nc.sync, 4)
            bk_sb = const.tile([128, NLIN], f32)
            nc.sync.dma_start(bk_sb[:], bk[:])
            nc.sync.dma_start(xqT_sb[:, 0:IBLK], xqT[:, 0:IBLK])
            nc.sync.dma_start(xqT_sb[:, IBLK:2 * IBLK], xqT[:, IBLK:2 * IBLK])
            kchunk_dma(nc.sync, 0)
            nc.sync.dma_start(t1b_sb[:], t1p[:].partition_broadcast(CHUNK))
            for c in (5, 1, 6, 2, 3):
                kchunk_dma(nc.sync, c)
            wq_sb = const.tile([128, 4, 128], bf16)
            nc.gpsimd.dma_start(wq_sb[:], wq[:].rearrange("p (l e) -> p l e", l=4))
            bq_sb = const.tile([128, 4], f32)
            nc.gpsimd.dma_start(bq_sb[:], bq[:])
            xt2b_sb = const.tile([128, max(NBSLOT, 1)], f32)
            nc.gpsimd.dma_start(xt2b_sb[:], xt2b[:])
            xkvB_sb = const.tile([128, max(NBSLOT, 1), 4], bf16)
            nc.gpsimd.dma_start(xkvB_sb[:], xkvB[:].rearrange("p (c f) -> p c f", f=4))
            probe_sb = const.tile([128, 64], bf16)
            nc.gpsimd.dma_start(probe_sb[:], probe[:])
            xkvD_sb = const.tile([128, NPAIR, 64], bf16)
            nc.gpsimd.dma_start(xkvD_sb[:], xkvD[:].rearrange("p (c f) -> p c f", f=64))
            step_sb = const.tile([64, 2 * NBLK], bf16)
            nc.gpsimd.dma_start(step_sb[:], stepm[:])
            outA_sb = const.tile([4, IBLK], f32)
            outB_sb = const.tile([4, IBLK], f32)

            # ---- blocked K^T target: pair tiles with block-diagonal layout
            kTblk = const.tile([128, NSLOT, CHUNK], bf16)
            nc.vector.memset(kTblk[0:64, :, 64:128], 0.0)
            nc.vector.memset(kTblk[64:128, :, 0:64], 0.0)
            qT2 = const.tile([128, NQ], bf16)
            G_all_sb = const.tile([64, 64], bf16)
            gstat = const.tile([128, NBLK, 4], bf16)
            nc.vector.memset(gstat[:], 0.0)

            # ---- stacked MLPs (block-diagonal weights, both halves at once)
            def epilogue(dst, ps, bias, layer, eng):
                if eng == "act":
                    func = Relu if layer < NLIN - 1 else Identity
                    nc.scalar.activation(dst, ps, func, bias=bias)
                elif layer < NLIN - 1:
                    nc.vector.tensor_scalar(dst, ps, bias, 0.0, op0=add, op1=amax)
                else:
                    nc.vector.tensor_scalar(dst, ps, bias, None, op0=add)

            hk1 = hkp.tile([128, KCOLS], bf16, tag="h")
            hk2 = hkp.tile([128, KCOLS], bf16, tag="h")
            hq1 = hqp.tile([128, NQ // 2], bf16, tag="h")
            hq2 = hqp.tile([128, NQ // 2], bf16, tag="h")

            def k_stage(c, layer, eng="act"):
                sl = slice(c * IBLK, min((c + 1) * IBLK, KCOLS))
                csz = sl.stop - sl.start
                cur = xkT_sb if layer == 0 else hk1
                dst = hk1 if layer == 0 else hk2
                ps = ps_a.tile([128, csz], f32, tag="a", name="ps")
                nc.tensor.matmul(
                    ps[:], wk_sb[:, layer, :], cur[:, sl], start=True, stop=True
                )
                epilogue(dst[:, sl], ps[:], bk_sb[:, layer:layer + 1], layer, eng)

            eng_flip = 0

            def k_final(c, act_only=False):
                nonlocal eng_flip
                sl = slice(c * IBLK, min((c + 1) * IBLK, KCOLS))
                csz = sl.stop - sl.start
                ps = ps_a.tile([128, csz], f32, tag="a", name="ps")
                nc.tensor.matmul(
                    ps[:], wk_sb[:, NLIN - 1, :], hk2[:, sl], start=True, stop=True
                )
                psv = ps[:].rearrange("p (a e) -> p a e", e=64)
                pair = slice(8 * c, 8 * c + csz // 64)
                bias = bk_sb[:, NLIN - 1 : NLIN]
                for half, csl in ((slice(0, 64), slice(0, 64)),
                                  (slice(64, 128), slice(64, 128))):
                    dst = kTblk[half, pair, csl]
                    srcv = psv[half, :, :]
                    if act_only or eng_flip % 2 == 0:
                        nc.scalar.activation(dst, srcv, Identity, bias=bias[half])
                    else:
                        nc.vector.tensor_scalar(dst, srcv, bias[half], None, op0=add)
                    eng_flip += 1

            def q_stage(layer, nb, eng="dve"):
                sl = slice(nb * IBLK, (nb + 1) * IBLK)
                cur = xqT_sb if layer == 0 else hq1
                dst = hq1 if layer == 0 else hq2
                ps = ps_a.tile([128, IBLK], f32, tag="a", name="ps")
                nc.tensor.matmul(
                    ps[:], wq_sb[:, layer, :], cur[:, sl], start=True, stop=True
                )
                epilogue(dst[:, sl], ps[:], bq_sb[:, layer:layer + 1], layer, eng)

            def q_final(nb, rep):
                sl = slice(nb * IBLK, (nb + 1) * IBLK)
                bias = bq_sb[:, NLIN - 1 : NLIN]
                ps = ps_a.tile([128, IBLK], f32, tag="a", name="ps")
                nc.tensor.matmul(
                    ps[:], wq_sb[:, 2 + rep, :], hq2[:, sl], start=True, stop=True
                )
                osl = slice(rep * (NQ // 2) + nb * IBLK,
                            rep * (NQ // 2) + (nb + 1) * IBLK)
                epilogue(qT2[:, osl], ps[:], bias, NLIN - 1,
                         "act" if rep else "dve")

            # ---- G machinery: two output banks (blocks 0-1 / 2-3)
            ovA = ps_ov.tile([4, IBLK], f32, tag="ov")
            ovB = ps_ov.tile([4, IBLK], f32, tag="ov")
            gall_ps = ps_g.tile([64, 64], f32, tag="g")
            sps_tiles = {}

            def job_probe_s(r):
                spb = ps_a.tile([128, 512], f32, tag="a", name="spb")
                for slq in range(8):
                    j = r * 8 + slq
                    nc.tensor.matmul(
                        spb[:, slq * 64:(slq + 1) * 64], kTblk[:, j, :],
                        probe_sb[:], start=True, stop=True, skip_group_check=True,
                    )
                sps = gpool.tile([128, 512], bf16, name="sps")
                nc.scalar.copy(sps[:], spb[:])
                sps_tiles[r] = sps

            def job_probe_av(r):
                sps = sps_tiles[r]
                for slq in range(8):
                    j = r * 8 + slq
                    nc.tensor.matmul(
                        gall_ps[:], xkvD_sb[:, j, :],
                        sps[:, slq * 64:(slq + 1) * 64],
                        start=(j == 0), stop=(j == NPAIR - 1),
                        skip_group_check=True,
                    )
                if r == NPAIR // 8 - 1:
                    nc.scalar.copy(G_all_sb[:], gall_ps[:])

            def job_select():
                psel = ps_g.tile([64, 2 * NBLK], f32, tag="g")
                nc.tensor.matmul(
                    psel[:], G_all_sb[:], step_sb[:],
                    start=True, stop=True, skip_group_check=True,
                )
                for b in range(NBLK):
                    for c in range(2):
                        i = c * NBLK + b
                        o = 2 * (b % 2) + c
                        nc.scalar.copy(gstat[0:64, b, o:o + 1],
                                       psel[:, i:i + 1])

            def job_full(b, stop=False):
                isl = slice(b * IBLK, (b + 1) * IBLK)
                ov = ovA if b < 2 else ovB
                nc.tensor.matmul(
                    ov[:], gstat[:, b, :], qT2[:, isl],
                    start=False, stop=stop, skip_group_check=True,
                )

            def readout(bank):
                ov = ovA if bank == 0 else ovB
                osb = outA_sb if bank == 0 else outB_sb
                nc.scalar.copy(osb[:], ov[:])
                for b in ((0, 1) if bank == 0 else (2, 3)):
                    isl = slice(b * IBLK, (b + 1) * IBLK)
                    eng = nc.sync if b % 2 == 0 else nc.gpsimd
                    o = 2 * (b % 2)
                    eng.dma_start(out[:, isl], osb[o:o + 2, :])

            # ---- HAM warm-up: dead matmuls on a memset stationary start
            # right after the preamble (no DMA dependency) so the PE clock is
            # at 8/8 when the real stream begins
            warmw = const.tile([128, 128], bf16)
            nc.vector.memset(warmw[:], 0.0)
            for w in range(24):
                wps = ps_s.tile([128, 128], f32, tag="s", name="wps")
                nc.tensor.matmul(
                    wps[:], warmw[:], warmw[:],
                    start=True, stop=True, skip_group_check=True,
                )

            # ---- critical chain: chunk 4 K + Q chain -> first S/mask asap
            k_stage(4, 0)
            q_stage(0, 0)
            q_stage(0, 1)
            k_stage(4, 1)
            q_stage(1, 0)
            q_stage(1, 1)
            k_final(4)
            q_final(0, 0)

            # ---- jobs: deadline list (sorted) + filler list, interleaved
            seq = [(b, s) for b in range(NBLK) for s in range(NBB[b])]

            def first_it_with_slot_ge(slot0):
                for i, (b, s) in enumerate(seq):
                    if boff[b] + s >= slot0:
                        return i
                return len(seq)

            def first_it_of_block(b0):
                for i, (b, s) in enumerate(seq):
                    if b == b0:
                        return i
                return len(seq)

            d5 = first_it_with_slot_ge(NPAIR + 8)
            d6 = first_it_with_slot_ge(NPAIR + 16)
            dB1 = first_it_of_block(1)
            dB2 = first_it_of_block(2)
            dB3 = first_it_of_block(3)

            dlist = sorted([
                (max(d5 - 2, 0), 0, lambda: k_stage(5, 0)),
                (max(d5 - 1, 0), 1, lambda: k_stage(5, 1)),
                (d5, 2, lambda: k_final(5)),
                (dB1, 3, lambda: q_final(1, 0)),
                (max(d6 - 2, 0), 4, lambda: k_stage(6, 0)),
                (max(d6 - 1, 0), 5, lambda: k_stage(6, 1)),
                (d6, 6, lambda: k_final(6, True)),
                (dB2, 7, lambda: q_final(0, 1)),
                (dB3, 8, lambda: q_final(1, 1)),
            ], key=lambda x: (x[0], x[1]))
            flist = [
                lambda: k_stage(0, 0),
                lambda: k_stage(0, 1),
                lambda: (k_final(0, True), job_probe_s(0)),
                lambda: k_stage(1, 0),
                lambda: k_stage(1, 1),
                lambda: (k_final(1, True), job_probe_s(1), job_probe_av(0)),
                lambda: k_stage(2, 0),
                lambda: k_stage(2, 1),
                lambda: (k_final(2, True), job_probe_s(2), job_probe_av(1)),
                lambda: k_stage(3, 0),
                lambda: k_stage(3, 1),
                lambda: (k_final(3, True), job_probe_s(3), job_probe_av(2)),
                lambda: job_probe_av(NPAIR // 8 - 1),
                lambda: job_select(),
                lambda: job_full(0),
                lambda: (job_full(1, stop=True), readout(0)),
                lambda: job_full(2),
                lambda: job_full(3),
            ]

            # ---- main interleave: jobs first, then S -> mask -> AV(deferred)
            first_av_b23 = next(i for i, (b, s) in enumerate(
                [(b, s) for b in range(NBLK) for s in range(NBB[b])]) if b == 2)

            def emit_av(b, s, first, last):
                slot = boff[b] + s
                bidx = slot - NPAIR
                ov = ovA if b < 2 else ovB
                nc.tensor.matmul(
                    ov[:], xkvB_sb[:, bidx, :], s_tiles[(b, s)][:],
                    start=first, stop=last, skip_group_check=True,
                )

            s_tiles = {}
            di = fi = 0
            prev_av = None
            for it, (b, s) in enumerate(seq):
                while di < len(dlist) and dlist[di][0] <= it:
                    dlist[di][2]()
                    di += 1
                while fi < len(flist) and di + fi < (it * 13) // 10 + 1:
                    flist[fi]()
                    fi += 1
                slot = boff[b] + s
                bidx = slot - NPAIR
                isl = slice(b * IBLK, (b + 1) * IBLK)
                sp = ps_s.tile([CHUNK, IBLK], f32, tag="s", name="sp")
                nc.tensor.matmul(
                    sp[:], kTblk[:, slot, :], qT2[:, isl],
                    start=True, stop=True, skip_group_check=True,
                )
                s_sb = spool.tile([CHUNK, IBLK], bf16, name="s_sb")
                nc.vector.scalar_tensor_tensor(
                    s_sb[:], t1b_sb[:, isl], xt2b_sb[:, bidx:bidx + 1], sp[:],
                    op0=is_ge, op1=mult,
                )
                s_tiles[(b, s)] = s_sb
                if prev_av is not None:
                    emit_av(*prev_av,
                            prev_av == seq[0] or prev_av == seq[first_av_b23],
                            False)
                prev_av = (b, s)
            while di < len(dlist):
                dlist[di][2]()
                di += 1
            while fi < len(flist):
                flist[fi]()
                fi += 1
            emit_av(*prev_av, False, True)
            readout(1)

    nc.compile()
    return nc


def kernel(x1, x2, x3, x4, Wq_w, Wq_b, Wk_w, Wk_b):
    from concourse.bass_utils import run_bass_kernel_spmd

    global LAST_RESULTS

    xs = [np.asarray(a, dtype=np.float32)[0, 0] for a in (x1, x2, x3, x4)]
    Wq_w = np.asarray(Wq_w, dtype=np.float32)
    Wq_b = np.asarray(Wq_b, dtype=np.float32)
    Wk_w = np.asarray(Wk_w, dtype=np.float32)
    Wk_b = np.asarray(Wk_b, dtype=np.float32)

    t1 = xs[0][:, -1]
    t2s = [x[:, -1] for x in xs]

    # ---- per-core full/boundary classification (exact, from timestamps)
    FJ = {}  # (m, p) -> (F[b], J[b])
    NBB = [1] * NBLK
    for p in range(2):
        qoff = NQ * p
        for m in range(M):
            F, J = [], []
            for b in range(NBLK):
                lo = t1[qoff + b * IBLK]
                hi = t1[qoff + b * IBLK + IBLK - 1]
                nfull = int(np.searchsorted(t2s[m], lo, side="right"))
                nvis = int(np.searchsorted(t2s[m], hi, side="right"))
                F.append(nfull // CHUNK)
                J.append(-(-nvis // CHUNK))
                NBB[b] = max(NBB[b], J[b] - F[b])
            FJ[(m, p)] = (F, J)

    nc = _build_program(NBB)

    NBSLOT = sum(NBB)
    boff = [sum(NBB[:b]) for b in range(NBLK)]

    # ---- host packing
    def blockdiag(Wl):
        b = np.zeros((128, 128), np.float32)
        b[:64, :64] = Wl
        b[64:, 64:] = Wl
        return b

    # Q weights: layers 0,1 blockdiag; final as [[W,W],[0,0]] and [[0,0],[W,W]]
    wq_h = np.zeros((4, 128, 128), np.float32)
    for l in range(NLIN - 1):
        wq_h[l] = blockdiag(Wq_w[l])
    wq_h[2, :64, :64] = Wq_w[2]
    wq_h[2, :64, 64:] = Wq_w[2]
    wq_h[3, 64:, :64] = Wq_w[2]
    wq_h[3, 64:, 64:] = Wq_w[2]
    wq_h = np.ascontiguousarray(wq_h.transpose(1, 0, 2).reshape(128, 4 * 128))
    bq_h = np.tile(Wq_b.T, (2, 1))  # [128, 3]
    bq_h = np.ascontiguousarray(
        np.concatenate([bq_h, bq_h[:, 2:3]], axis=1)
    )  # [128, 4]

    probe_h = np.ascontiguousarray(
        np.concatenate([np.eye(64, dtype=np.float32)] * 2, axis=0)
    )  # [128, 64]

    x1T = np.ascontiguousarray(xs[0].T)

    def pack_tile(xrows):
        """[128, D] key rows -> ([128, 64] xkT block, [128, 2] V, [128] t2)."""
        ev, od = xrows[0:64], xrows[64:128]
        blk = np.concatenate([ev.T, od.T], axis=0)  # [128, 64]
        v = np.concatenate([ev[:, 0:2], od[:, 0:2]], axis=0)  # [128, 2]
        tt = np.concatenate([ev[:, -1], od[:, -1]], axis=0)  # [128]
        return blk, v, tt

    in_maps = []
    for c in range(8):
        m, p = c // 2, c % 2
        xm = xs[m]
        qoff = NQ * p
        F, J = FJ[(m, p)]

        NSLOT = NPAIR + NBSLOT
        xkT_h = np.zeros((128, NSLOT * 64), np.float32)
        xkvD_h = np.zeros((128, NPAIR, 64), np.float32)
        xkvB_h = np.zeros((128, max(NBSLOT, 1), 4), np.float32)
        xt2b_h = np.full((128, max(NBSLOT, 1)), 1e30, np.float32)
        for j in range(NPAIR):
            blk, v, tt = pack_tile(xm[CHUNK * j:CHUNK * (j + 1)])
            xkT_h[:, 64 * j:64 * (j + 1)] = blk
            xkvD_h[:, j, 2 * j: 2 * j + 2] = v
        for b in range(NBLK):
            for s in range(NBB[b]):
                t = F[b] + s
                slot = NPAIR + boff[b] + s
                if t < J[b]:
                    blk, v, tt = pack_tile(xm[CHUNK * t:CHUNK * (t + 1)])
                    xkT_h[:, 64 * slot:64 * (slot + 1)] = blk
                    if not os.environ.get('DBG_NO_BND'):
                        o = 2 * (b % 2)
                        xkvB_h[:, boff[b] + s, o:o + 2] = v
                    xt2b_h[:, boff[b] + s] = tt
        step_h = np.zeros((64, 2 * NBLK), np.float32)
        if not os.environ.get('DBG_NO_FULL'):
            for b in range(NBLK):
                for c in range(2):
                    for j in range(F[b]):
                        step_h[2 * j + c, c * NBLK + b] = 1.0

        wk_h = np.stack([blockdiag(Wk_w[m][l]) for l in range(NLIN)])
        wk_h = np.ascontiguousarray(wk_h.transpose(1, 0, 2).reshape(128, NLIN * 128))
        bk_h = np.ascontiguousarray(np.tile(Wk_b[m].T, (2, 1)))  # [128, 3]

        # query-side: contiguous half, [first 1024 | second 1024] stacking
        xq = x1T[:, qoff:qoff + NQ]  # [64, 2048]
        xqT_h = np.concatenate([xq[:, : NQ // 2], xq[:, NQ // 2:]], axis=0)

        in_maps.append(
            {
                "xqT": np.ascontiguousarray(xqT_h).astype(BF16),
                "xkT": xkT_h.astype(BF16),
                "xkvD": np.ascontiguousarray(xkvD_h.reshape(128, NPAIR * 64)).astype(BF16),
                "xkvB": np.ascontiguousarray(
                    xkvB_h.reshape(128, max(NBSLOT, 1) * 4)).astype(BF16),
                "xt2b": xt2b_h,
                "t1p": np.ascontiguousarray(t1[qoff:qoff + NQ][None, :]),
                "probe": probe_h.astype(BF16),
                "stepm": step_h.astype(BF16),
                "wq": wq_h.astype(BF16),
                "bq": bq_h,
                "wk": wk_h.astype(BF16),
                "bk": bk_h,
            }
        )

    res = run_bass_kernel_spmd(nc, in_maps, core_ids=list(range(8)))
    LAST_RESULTS = res

    # ---- gather: sum over modalities per contiguous half, transpose
    acc = np.zeros((2, T), dtype=np.float32)
    for c in range(8):
        m, p = c // 2, c % 2
        acc[:, NQ * p:NQ * (p + 1)] += res.results[c]["out"]
    return np.ascontiguousarray(acc.T)[None]
